# revision 28
# baseline (speedup 1.0000x reference)
"""MLA decode kernel for Trainium2, data-parallel over batch across 8 NeuronCores.

Each core handles 4 batches. Host prep (numpy only — layout/dtype/swizzle, no
model math):
  - cast weights/cache to bf16
  - pre-swizzle all large tensors into [128-partition, contiguous-run] blocked
    layouts so every big DMA is 128 descriptors of multi-KB contiguous runs
    (HWDGE descriptor generation costs the issuing engine ~6ns/descriptor, so
    1KB-chunk layouts waste tens of us of engine time)
  - kvpe_sw [b, n, 128, 4, 1024]: kv_cache^T tiles (c on partitions, for the
    scores matmul); pe_sw [b, n, 64, 1024]: pe_cache^T tiles
  - kvnat_sw [b, n, 128, 8, 512]: natural-layout tiles (t on partitions, for
    the PV matmul)
On-device: q/kv projections, rms_norm, rope, weight-absorbed MLA attention with
a fully streaming softmax (no max-subtraction: logits are ~N(0,1) by
construction, exp is safe in f32), PV accumulated in per-batch PSUM banks,
v-projection, output projection. Cache rows at start_pos..start_pos+4 are
replaced on-chip with the freshly projected values (reference semantics).

Pipeline: per 512-position score step the PE stream is
  [5x scores MM (i)] [4x PV MM (i-1)] [4x P-transpose (i)]
so exp(i) (ACT) hides under PV(i-1) and the P^T copy (DVE) hides under the
next step's scores. PV uses 4 persistent PSUM banks (one per batch); per-batch
normalization/transposes are deferred a few steps so they never stall the PE.
"""

import os
import sys

sys.path.insert(0, "/opt/trn_rl_repo")

import numpy as np
import ml_dtypes

import concourse.bass as bass
import concourse.bacc as bacc_mod
import concourse.mybir as mybir
from concourse.bass_utils import run_bass_kernel_spmd
from concourse.masks import make_identity
from concourse.tile import TileContext

BF16 = mybir.dt.bfloat16
F32 = mybir.dt.float32
NBF = ml_dtypes.bfloat16

DIM = 2048
N_HEADS = 16
Q_LORA = 1536
KV_LORA = 512
QK_NOPE = 128
QK_ROPE = 64
V_DIM = 128
QK_HD = QK_NOPE + QK_ROPE  # 192
MAX_SEQ = 8192
BSZ = 32
SEQLEN = 4
START_POS = MAX_SEQ - SEQLEN
EPS = 1e-6
SCALE = QK_HD ** -0.5

N_CORES = 8
BPC = BSZ // N_CORES          # batches per core = 4
M = BPC * SEQLEN              # rows per core = 16 (b, s)
NT = 1024                     # t-positions per DMA tile
N_NTILES = MAX_SEQ // NT      # 8
HALVES = 2                    # 512-t score steps per DMA tile
CROWS = KV_LORA + QK_ROPE     # 576 rows of kvpeT


def build_bass():
    nc = bacc_mod.Bacc(target_bir_lowering=False)

    xTs = nc.dram_tensor("xTs", [128, DIM // 128, M], BF16, kind="ExternalInput")
    wqa = nc.dram_tensor("wqa", [3, 128, 16, 512], BF16, kind="ExternalInput")
    wqb = nc.dram_tensor("wqb", [6, 128, 12, 512], BF16, kind="ExternalInput")
    wkva = nc.dram_tensor("wkva", [128, 16, KV_LORA + QK_ROPE], BF16, kind="ExternalInput")
    wkvb_nope = nc.dram_tensor("wkvb_nope", [128, N_HEADS, KV_LORA], BF16, kind="ExternalInput")
    wkvb_vT = nc.dram_tensor("wkvb_vT", [128, N_HEADS, KV_LORA // 128, V_DIM], BF16, kind="ExternalInput")
    wo = nc.dram_tensor("wo", [4, 128, 16, 512], BF16, kind="ExternalInput")
    qnw = nc.dram_tensor("qnw", [M, Q_LORA], BF16, kind="ExternalInput")
    kvnw = nc.dram_tensor("kvnw", [M, KV_LORA], BF16, kind="ExternalInput")
    cosq = nc.dram_tensor("cosq", [M, N_HEADS, QK_ROPE // 2], BF16, kind="ExternalInput")
    sinq = nc.dram_tensor("sinq", [M, N_HEADS, QK_ROPE // 2], BF16, kind="ExternalInput")
    cosk = nc.dram_tensor("cosk", [M, QK_ROPE // 2], F32, kind="ExternalInput")
    sink = nc.dram_tensor("sink", [M, QK_ROPE // 2], F32, kind="ExternalInput")
    kvpe_sw = nc.dram_tensor("kvpe_sw", [BPC, N_NTILES, 128, 4, NT], BF16, kind="ExternalInput")
    pe_sw = nc.dram_tensor("pe_sw", [BPC, N_NTILES, 64, NT], BF16, kind="ExternalInput")
    kvnat_sw = nc.dram_tensor("kvnat_sw", [BPC, N_NTILES, 128, NT // 128, KV_LORA], BF16, kind="ExternalInput")
    out = nc.dram_tensor("out", [M, DIM], F32, kind="ExternalOutput")

    with TileContext(nc) as tc:
        with (
            tc.tile_pool(name="const", bufs=1) as cpool,
            tc.tile_pool(name="wstream", bufs=2) as wpool,
            tc.tile_pool(name="proj", bufs=1) as ppool,
            tc.tile_pool(name="ps_a", bufs=2, space="PSUM") as ps_a,      # proj + scores psums
            tc.tile_pool(name="ps_pv", bufs=1, space="PSUM") as ps_pv,    # 4 tags -> 4 banks
            tc.tile_pool(name="ps_t", bufs=2, space="PSUM") as ps_t,      # transposes / small mms
            tc.tile_pool(name="ktile", bufs=3) as ktpool,
            tc.tile_pool(name="kvtile", bufs=3) as kvpool,
            tc.tile_pool(name="sP", bufs=2) as ppoolP,
            tc.tile_pool(name="sPT", bufs=2) as ptpool,
            tc.tile_pool(name="sMisc", bufs=2) as spool_m,
        ):
            ident = cpool.tile([128, 128], BF16)
            make_identity(nc, ident)
            eps_sb = cpool.tile([M, 1], F32)
            nc.gpsimd.memset(eps_sb, EPS)

            # ---------- first matmul deps first (scalar HWDGE queue) ----------
            xT_sb = cpool.tile([128, DIM // 128, M], BF16)
            nc.scalar.dma_start(xT_sb, xTs[:, :, :])

            # ---------- q1 = x @ wq_a ; kvfull = x @ wkv_a ----------
            nkt = DIM // 128

            def proj_chunk(w_sb, n_lo, n_hi, out_sb):
                nn = n_hi - n_lo
                ps = ps_a.tile([M, 512], F32, tag="psA")
                for k in range(nkt):
                    nc.tensor.matmul(
                        ps[:, :nn], xT_sb[:, k, :], w_sb[:, k, n_lo:n_hi],
                        start=(k == 0), stop=(k == nkt - 1),
                    )
                nc.vector.tensor_copy(out_sb[:, :nn], ps[:, :nn])

            q1 = ppool.tile([M, Q_LORA], F32, tag="big1")
            for c in range(3):
                w_sb = wpool.tile([128, 16, 512], BF16, tag="wproj")
                nc.scalar.dma_start(w_sb, wqa[c])
                proj_chunk(w_sb, 0, 512, q1[:, c * 512:(c + 1) * 512])

            # small residents (needed a bit later than wqa chunk 0)
            qnw_sb = cpool.tile([M, Q_LORA], BF16)
            nc.scalar.dma_start(qnw_sb, qnw[:, :])
            kvnw_sb = cpool.tile([M, KV_LORA], BF16)
            nc.scalar.dma_start(kvnw_sb, kvnw[:, :])
            cosq_sb = cpool.tile([M, N_HEADS, QK_ROPE // 2], BF16)
            nc.scalar.dma_start(cosq_sb, cosq[:, :, :])
            sinq_sb = cpool.tile([M, N_HEADS, QK_ROPE // 2], BF16)
            nc.scalar.dma_start(sinq_sb, sinq[:, :, :])
            cosk_sb = cpool.tile([M, QK_ROPE // 2], F32)
            nc.scalar.dma_start(cosk_sb, cosk[:, :])
            sink_sb = cpool.tile([M, QK_ROPE // 2], F32)
            nc.scalar.dma_start(sink_sb, sink[:, :])

            kvfull = ppool.tile([M, KV_LORA + QK_ROPE], F32, tag="big2")
            wkva_sb = wpool.tile([128, 16, 576], BF16, tag="wproj")
            nc.scalar.dma_start(wkva_sb, wkva[:, :, :])
            proj_chunk(wkva_sb, 0, 512, kvfull[:, 0:512])
            proj_chunk(wkva_sb, 512, 576, kvfull[:, 512:576])

            def rms_norm_cast(x_sb, n, w_sb, out_bf):
                ss = ppool.tile([M, 1], F32, tag="rms_ss")
                tmp = ppool.tile([M, n], F32, tag="big3")
                nc.scalar.activation(
                    out=tmp, in_=x_sb[:, :n],
                    func=mybir.ActivationFunctionType.Square, accum_out=ss,
                )
                rstd = ppool.tile([M, 1], F32, tag="rms_rstd")
                nc.scalar.activation(
                    out=rstd, in_=ss, func=mybir.ActivationFunctionType.Sqrt,
                    scale=1.0 / n, bias=eps_sb,
                )
                nc.vector.reciprocal(rstd, rstd)
                nc.vector.tensor_scalar_mul(tmp, x_sb[:, :n], rstd)
                nc.vector.tensor_tensor(out_bf, tmp, w_sb, op=mybir.AluOpType.mult)

            q1n = ppool.tile([M, Q_LORA], BF16)
            rms_norm_cast(q1, Q_LORA, qnw_sb, q1n)
            kvlat = ppool.tile([M, KV_LORA], BF16)
            rms_norm_cast(kvfull, KV_LORA, kvnw_sb, kvlat)

            def rope(e, o, cos, sin, oe, oo):
                t1 = ppool.tile(list(e.shape), F32, tag="rope_t1")
                t2 = ppool.tile(list(e.shape), F32, tag="rope_t2")
                nc.vector.tensor_tensor(t1, e, cos, op=mybir.AluOpType.mult)
                nc.vector.tensor_tensor(t2, o, sin, op=mybir.AluOpType.mult)
                nc.vector.tensor_tensor(oe, t1, t2, op=mybir.AluOpType.subtract)
                nc.vector.tensor_tensor(t1, e, sin, op=mybir.AluOpType.mult)
                nc.vector.tensor_tensor(t2, o, cos, op=mybir.AluOpType.mult)
                nc.vector.tensor_tensor(oo, t1, t2, op=mybir.AluOpType.add)

            # k_pe rope -> bf16 [M, 64]
            kpe = ppool.tile([M, QK_ROPE], BF16)
            kvf_pairs = kvfull[:, KV_LORA:KV_LORA + QK_ROPE].rearrange("p (a two) -> p a two", two=2)
            kpe_pairs = kpe.rearrange("p (a two) -> p a two", two=2)
            rope(kvf_pairs[:, :, 0], kvf_pairs[:, :, 1], cosk_sb, sink_sb,
                 kpe_pairs[:, :, 0], kpe_pairs[:, :, 1])

            # ---------- q = q1n @ wq_b  (need q1n^T as lhsT) ----------
            q1nT = ppool.tile([128, Q_LORA // 128, M], BF16)
            for k in range(Q_LORA // 128):
                pt = ps_t.tile([128, M], BF16, tag="tps")
                nc.tensor.transpose(pt, q1n[:, k * 128:(k + 1) * 128], ident[:M, :M])
                nc.vector.tensor_copy(q1nT[:, k, :], pt)

            q = ppool.tile([M, N_HEADS, QK_HD], BF16, tag="big1")
            qf = q.rearrange("p h d -> p (h d)")
            for c in range(6):
                w_sb = wpool.tile([128, 12, 512], BF16, tag="wproj")
                nc.scalar.dma_start(w_sb, wqb[c])
                ps = ps_a.tile([M, 512], F32, tag="psA")
                for k in range(Q_LORA // 128):
                    nc.tensor.matmul(ps, q1nT[:, k, :], w_sb[:, k, :],
                                     start=(k == 0), stop=(k == Q_LORA // 128 - 1))
                nc.vector.tensor_copy(qf[:, c * 512:(c + 1) * 512], ps)

            # rope q_pe (all heads at once) -> bf16, and cast q_nope -> bf16
            qpe = ppool.tile([M, N_HEADS, QK_ROPE], BF16)
            q_pairs = q[:, :, QK_NOPE:].rearrange("p h (a two) -> p h a two", two=2)
            qpe_pairs = qpe.rearrange("p h (a two) -> p h a two", two=2)
            rope(q_pairs[:, :, :, 0], q_pairs[:, :, :, 1], cosq_sb, sinq_sb,
                 qpe_pairs[:, :, :, 0], qpe_pairs[:, :, :, 1])
            qnope = ppool.tile([M, N_HEADS, QK_NOPE], BF16, tag="big3")
            nc.vector.tensor_copy(qnope, q[:, :, :QK_NOPE])

            # ---------- absorbed QT [128, 5, b, 64]: j=0..3 = (wkvb_nope^T qnope^T), j=4 = qpe^T ----------
            wkvbn_sb = cpool.tile([128, N_HEADS, KV_LORA], BF16, tag="wkvb")
            nc.scalar.dma_start(wkvbn_sb, wkvb_nope[:, :, :])
            QT = ppool.tile([128, 5, BPC, 64], BF16)
            for h in range(N_HEADS):
                pt = ps_t.tile([128, M], BF16, tag="tps")
                nc.tensor.transpose(pt, qnope[:, h, :], ident[:M, :M])
                qnT_h = ppool.tile([128, M], BF16, tag="qnTh")
                nc.vector.tensor_copy(qnT_h, pt)
                for c in range(KV_LORA // 128):
                    pa = ps_t.tile([128, M], F32, tag="tps")
                    nc.tensor.matmul(pa, wkvbn_sb[:, h, c * 128:(c + 1) * 128], qnT_h,
                                     start=True, stop=True)
                    nc.vector.tensor_copy(
                        QT[:, c, :, h * SEQLEN:(h + 1) * SEQLEN],
                        pa.rearrange("p (b s) -> p b s", b=BPC))
                ptp = ps_t.tile([64, M], BF16, tag="tps")
                nc.tensor.transpose(ptp, qpe[:, h, :], ident[:M, :M])
                nc.vector.tensor_copy(
                    QT[:64, 4, :, h * SEQLEN:(h + 1) * SEQLEN],
                    ptp.rearrange("p (b s) -> p b s", b=BPC))

            # v-proj weights: aliases wkvb_nope's SBUF (absorb is done with it);
            # gpsimd queue so it doesn't block the scalar kv8 stream
            wkvbv_sb = cpool.tile([128, N_HEADS, KV_LORA // 128, V_DIM], BF16, tag="wkvb")
            nc.gpsimd.dma_start(wkvbv_sb, wkvb_vT[:, :, :, :])

            # ---------- transposed new cache rows ----------
            kvlatT = ppool.tile([128, KV_LORA // 128, M], BF16)
            for k in range(KV_LORA // 128):
                pt = ps_t.tile([128, M], BF16, tag="tps")
                nc.tensor.transpose(pt, kvlat[:, k * 128:(k + 1) * 128], ident[:M, :M])
                nc.vector.tensor_copy(kvlatT[:, k, :], pt)
            kpeT = ppool.tile([64, M], BF16)
            ptp = ps_t.tile([64, M], BF16, tag="tps")
            nc.tensor.transpose(ptp, kpe, ident[:M, :M])
            nc.vector.tensor_copy(kpeT, ptp)

            # ---------- streaming attention ----------
            ssum_parts = cpool.tile([64, BPC, N_NTILES * HALVES], F32)
            pv_banks = {}
            kv_last = {}
            for b in range(BPC):
                pv_banks[b] = ps_pv.tile([64, KV_LORA], F32, tag=f"pv{b}",
                                         name=f"pv_bank{b}")
            outT = ppool.tile([128, KV_LORA // 128, N_HEADS, M], BF16)

            def emit_pv(state):
                b, n, half, PTs, kv8 = state
                po = pv_banks[b]
                for i in range(4):
                    k = (n * HALVES + half) * 4 + i
                    rhs = kv8[:, half * 4 + i, :]
                    if n == N_NTILES - 1 and half == 1 and i == 3:
                        rhs = kv_last[b][:, :]
                    nc.tensor.matmul(
                        po, PTs[:, i, :], rhs,
                        start=(k == 0), stop=(k == N_NTILES * HALVES * 4 - 1),
                    )

            ob_tiles = {}

            def emit_norm(b):
                """normalize pv bank b -> ob (DVE chain)"""
                ssum = spool_m.tile([64, 1], F32, tag="ssum")
                nc.vector.tensor_reduce(ssum, ssum_parts[:, b, :],
                                        axis=mybir.AxisListType.X, op=mybir.AluOpType.add)
                rsum = spool_m.tile([64, 1], F32, tag="rsum")
                nc.vector.reciprocal(rsum, ssum)
                ob = spool_m.tile([64, KV_LORA], BF16, tag="ob")
                nc.vector.tensor_scalar_mul(ob, pv_banks[b], rsum)
                ob_tiles[b] = ob

            def emit_outT(b):
                ob = ob_tiles[b]
                for c in range(KV_LORA // 128):
                    pt = ps_t.tile([128, 64], BF16, tag="tps")
                    nc.tensor.transpose(pt, ob[:, c * 128:(c + 1) * 128], ident[:64, :64])
                    nc.vector.tensor_copy(
                        outT[:, c, :, b * SEQLEN:(b + 1) * SEQLEN],
                        pt.rearrange("p (h s) -> p h s", h=N_HEADS))

            prev = None          # (b, n, half, PTs, kv8)
            deferred = []        # (due_step, fn)
            step_idx = 0
            for b in range(BPC):
                for n in range(N_NTILES):
                    if n == 0:
                        # last 128-t chunk, loaded early and patched with the
                        # new rows (t = 8188..8191 -> partitions 124..127)
                        kl = kvpool.tile([128, KV_LORA], BF16, tag=f"kvlast{b}",
                                         name=f"kv_last{b}", bufs=1)
                        nc.scalar.dma_start(kl, kvnat_sw[b, N_NTILES - 1, :, NT // 128 - 1, :])
                        nc.gpsimd.dma_start(
                            kl[124:128, :], kvlat[b * SEQLEN:(b + 1) * SEQLEN, :])
                        kv_last[b] = kl
                    nchunk = NT // 128 if n < N_NTILES - 1 else NT // 128 - 1
                    # first tiles issue on scalar: their DMAs then queue FIFO
                    # behind the weight streams so the prologue weights get
                    # full HBM bandwidth; later tiles (sync queue) are gated
                    # by pool-buffer release anyway.
                    kt_eng = nc.scalar if (b * N_NTILES + n) < 4 else nc.sync
                    kt_main = ktpool.tile([128, 4, NT], BF16, tag="ktm", bufs=4)
                    kt_eng.dma_start(kt_main, kvpe_sw[b, n])
                    kt_pe = ktpool.tile([64, NT], BF16, tag="ktpe", bufs=4)
                    kt_eng.dma_start(kt_pe, pe_sw[b, n])
                    kv8 = kvpool.tile([128, NT // 128, KV_LORA], BF16, tag="kvnat", bufs=4)
                    nc.scalar.dma_start(kv8[:, :nchunk, :], kvnat_sw[b, n, :, :nchunk, :])
                    if n == N_NTILES - 1:
                        # patch the 4 new rows in the score operands
                        nc.vector.tensor_copy(
                            kt_main[:, :, NT - SEQLEN:],
                            kvlatT[:, :, b * SEQLEN:(b + 1) * SEQLEN])
                        nc.vector.tensor_copy(
                            kt_pe[:, NT - SEQLEN:], kpeT[:, b * SEQLEN:(b + 1) * SEQLEN])
                    for half in range(HALVES):
                        c0 = half * 512
                        S = ps_a.tile([64, 512], F32, tag="psA")
                        for j in range(4):
                            nc.tensor.matmul(S, QT[:, j, b, :], kt_main[:, j, c0:c0 + 512],
                                             start=(j == 0), stop=False)
                        nc.tensor.matmul(S, QT[:64, 4, b, :], kt_pe[:, c0:c0 + 512],
                                         start=False, stop=True)
                        if prev is not None:
                            emit_pv(prev)
                            if prev[0] != b:
                                deferred.append((step_idx + 1, lambda pb=prev[0]: emit_norm(pb)))
                                deferred.append((step_idx + 3, lambda pb=prev[0]: emit_outT(pb)))
                        for due, fn in [d for d in deferred if d[0] <= step_idx]:
                            fn()
                        deferred = [d for d in deferred if d[0] > step_idx]
                        P = ppoolP.tile([64, 512], BF16, tag="P")
                        col = n * HALVES + half
                        nc.scalar.activation(
                            out=P, in_=S, func=mybir.ActivationFunctionType.Exp,
                            scale=SCALE, accum_out=ssum_parts[:, b, col:col + 1])
                        ptr = ps_t.tile([128, 4, 64], BF16, tag="tps")
                        for i in range(4):
                            nc.tensor.transpose(ptr[:, i, :], P[:, i * 128:(i + 1) * 128],
                                                ident[:64, :64])
                        PTs = ptpool.tile([128, 4, 64], BF16, tag="PT")
                        nc.vector.tensor_copy(PTs, ptr)
                        prev = (b, n, half, PTs, kv8)
                        step_idx += 1
            emit_pv(prev)
            for _, fn in deferred:
                fn()
            emit_norm(BPC - 1)
            emit_outT(BPC - 1)

            # ---------- v-proj: o2T[d, h, M] = wkvb_vT^T @ outT ----------
            o2T = ppool.tile([128, N_HEADS, M], BF16)
            for h in range(N_HEADS):
                pv = ps_t.tile([128, M], F32, tag="tps")
                for k in range(KV_LORA // 128):
                    nc.tensor.matmul(
                        pv, wkvbv_sb[:, h, k, :],
                        outT[:, k, h, :],
                        start=(k == 0), stop=(k == KV_LORA // 128 - 1),
                    )
                nc.vector.tensor_copy(o2T[:, h, :], pv)

            # ---------- final: out = o2 @ wo (stream wo on sync queue) ----------
            fin = ppool.tile([M, DIM], F32, tag="big1")
            for c in range(4):
                w_sb = wpool.tile([128, 16, 512], BF16, tag="wo")
                nc.sync.dma_start(w_sb, wo[c])
                pf = ps_a.tile([M, 512], F32, tag="psA")
                for h in range(N_HEADS):
                    nc.tensor.matmul(pf, o2T[:, h, :], w_sb[:, h, :],
                                     start=(h == 0), stop=(h == N_HEADS - 1))
                nc.vector.tensor_copy(fin[:, c * 512:(c + 1) * 512], pf)
            nc.sync.dma_start(out[:, :], fin)

    nc.compile()
    return nc


_NC_CACHE = {}


def kernel(x, wq_a, q_norm_w, wq_b, wkv_a, kv_norm_w, wkv_b, wo,
           kv_cache, pe_cache, freqs_cos, freqs_sin, start_pos):
    assert int(start_pos) == START_POS
    bf = lambda a: np.ascontiguousarray(np.asarray(a), dtype=NBF)
    f32 = lambda a: np.ascontiguousarray(np.asarray(a), dtype=np.float32)
    C = np.ascontiguousarray

    x = f32(x)
    wkv_b_r = f32(wkv_b).reshape(N_HEADS, QK_NOPE + V_DIM, KV_LORA)
    # [h, 128, 512] -> [128(p), h, 512]
    wkvbn_sw = C(bf(wkv_b_r[:, :QK_NOPE, :]).transpose(1, 0, 2))
    # [h, 512, 128] -> [128(p of c-chunk), h, 4, 128]
    wkvbv_sw = C(bf(np.swapaxes(wkv_b_r[:, QK_NOPE:, :], 1, 2))
                 .reshape(N_HEADS, 4, 128, V_DIM).transpose(2, 0, 1, 3))
    wqa_sw = C(bf(wq_a).reshape(16, 128, 3, 512).transpose(2, 1, 0, 3))
    wqb_sw = C(bf(wq_b).reshape(12, 128, 6, 512).transpose(2, 1, 0, 3))
    wkva_sw = C(bf(wkv_a).reshape(16, 128, 576).transpose(1, 0, 2))
    wo_sw = C(bf(wo).reshape(16, 128, 4, 512).transpose(2, 1, 0, 3))

    cos = f32(freqs_cos); sin = f32(freqs_sin)                   # [4, 32]
    cosM = np.tile(cos, (BPC, 1))                                # [16, 32]
    sinM = np.tile(sin, (BPC, 1))
    cosq = np.repeat(cosM[:, None, :], N_HEADS, axis=1)          # [16, 16, 32]
    sinq = np.repeat(sinM[:, None, :], N_HEADS, axis=1)
    qnw = np.tile(bf(q_norm_w)[None, :], (M, 1))
    kvnw = np.tile(bf(kv_norm_w)[None, :], (M, 1))

    kv_bf = bf(kv_cache)                                         # [32, 8192, 512]
    pe_bf = bf(pe_cache)                                         # [32, 8192, 64]

    in_maps = []
    for c in range(N_CORES):
        bs = slice(c * BPC, (c + 1) * BPC)
        kvc = kv_bf[bs]                                          # [4, 8192, 512]
        pec = pe_bf[bs]                                          # [4, 8192, 64]
        # [b, n, p(c-chunk), j, nt]: kvpe_sw[b,n,p,j,:] = kv[b, n*NT+nt, j*128+p]
        kvpe_sw = C(kvc.reshape(BPC, N_NTILES, NT, 4, 128).transpose(0, 1, 4, 3, 2))
        pe_sw = C(pec.reshape(BPC, N_NTILES, NT, 64).transpose(0, 1, 3, 2))
        # [b, n, p(t in chunk), g, c]
        kvnat_sw = C(kvc.reshape(BPC, N_NTILES, NT // 128, 128, KV_LORA)
                     .transpose(0, 1, 3, 2, 4))
        xc = bf(x[bs].reshape(M, DIM).T)                         # [2048, 16]
        xTs = C(xc.reshape(16, 128, M).transpose(1, 0, 2))       # [128, 16, 16]
        in_maps.append({
            "xTs": xTs,
            "wqa": wqa_sw, "wqb": wqb_sw, "wkva": wkva_sw,
            "wkvb_nope": wkvbn_sw, "wkvb_vT": wkvbv_sw, "wo": wo_sw,
            "qnw": qnw, "kvnw": kvnw,
            "cosq": C(cosq.astype(NBF)), "sinq": C(sinq.astype(NBF)),
            "cosk": C(cosM), "sink": C(sinM),
            "kvpe_sw": kvpe_sw, "pe_sw": pe_sw, "kvnat_sw": kvnat_sw,
        })

    if "nc" not in _NC_CACHE:
        _NC_CACHE["nc"] = build_bass()
    nc = _NC_CACHE["nc"]

    trace = os.environ.get("KERNEL_TRACE", "0") == "1"
    res = run_bass_kernel_spmd(nc, in_maps, core_ids=list(range(N_CORES)), trace=trace)
    _NC_CACHE["res"] = res
    if trace and res.exec_time_ns is not None:
        print(f"HW exec time: {res.exec_time_ns} ns")
        _NC_CACHE["last_exec_ns"] = res.exec_time_ns

    outs = [r["out"].reshape(BPC, SEQLEN, DIM) for r in res.results]
    return np.concatenate(outs, axis=0).astype(np.float32)
